# revision 6
# baseline (speedup 1.0000x reference)
"""FFTEmbedding kernel for Trainium2 (8 NeuronCores, SPMD data-parallel over B).

Math: the reference computes, per (b, t):
    window = x_pad[b, t : t+W]                (causal window, W=256)
    spec   = rfft(window); feats = [spec.real, spec.imag]   (258)
    out    = feats @ weight.T + bias          (512)

The pipeline is linear in x, so it collapses to a causal 1-D convolution
with a precomputed (W=256, EMB=512) matrix:
    M2[w, e] = sum_k weight[e, k]*cos(2*pi*k*w/W) - weight[e, 129+k]*sin(2*pi*k*w/W)
    out[b, t, e] = sum_w x_pad[b, t+w] * M2[w, e] + bias[e]

Device mapping (per core: 2 batch rows, weights replicated). The whole
kernel is DMA-throughput-bound (~255 GB/s/core aggregate across the 16
shared DMA engines; per-packet rate caps at ~21-26 GB/s/engine and small
runs are far worse), so the design optimizes BYTES and RUN LENGTH:
  * "mega-Hankel" SBUF image Hank[p, c] = x_pad[b, p+c], materialized by
    DMAs whose source access pattern overlaps: partition p reads the run
    x_pad[b, p : p+...]. A 128-column slice IS the pre-transposed lhsT.
    Chunks are >=1536 cols so per-partition runs are >=3 KB (512-byte
    runs measured ~16 GB/s -- never use tiny chunks).
  * per 128-t output tile i: psum[128, 512] = Hank[:, 128i:+128].T @ W0
    + Hank[:, 128(i+1):+128].T @ W1   (fp16 operands, fp32 PSUM accum).
    Two tiles share one 2-bank PSUM pair, evacuated with bias fused:
    DVE tensor_tensor, with ACT copy + 16-bit DVE add on 3 of 5 pairs
    (GPSIMD tensor_tensor measured ~5us per pair -- never use it).
  * output uses a PARTITION-MAJOR DRAM layout out_dev[b, p, c*512+e] =
    out[b, 128c+p, e]: each supertile store is a plain 2D slice with an
    8 KB contiguous run per partition (vs 4x1 KB strided runs for the
    natural layout). The host un-permutes with a cheap numpy reshape.
  * input chunks and output supertiles alternate between the sync and
    scalar HWDGE rings (separate FIFOs, shared engine pool); the final
    supertile is stored as two halves, one per ring, to cut the drain.
  * PE warm-up: HAM clock gate needs ~3.4us of sustained PE activity; a
    run of N=128 junk matmuls bridges the input-DMA wait so the real
    stream starts at the warm clock rate.
  * output staged fp16 in SBUF, stored fp16 (halves the dominant HBM
    write stream); host upcasts to fp32. End-to-end rel err ~4e-4.
"""

import os
import sys

import numpy as np

_TRN_REPO = "/opt/trn_rl_repo"
if _TRN_REPO not in sys.path:
    sys.path.insert(0, _TRN_REPO)

B, T, W_SIZE, EMB = 16, 8192, 256, 512
N_CORES = 8
B_PER = B // N_CORES          # 2 batch rows per core
PAD = W_SIZE - 1              # 255 leading zeros
XP_LEN = T + PAD + 1          # 8448 (one trailing pad elem)
HANK_COLS = T + W_SIZE - 128  # 8320 mega-Hankel free dim
N_TILES = T // 128            # 64 output tiles of 128 t's per batch row
N_PAIRS = N_TILES // 2        # 32 PSUM pairs per batch row
PAIRS_PER_SUP = 4             # supertile = 4 pairs = 8 tiles = 1024 t rows
N_SUP = N_PAIRS // PAIRS_PER_SUP
SUP_W = 2 * PAIRS_PER_SUP * EMB  # 4096 fp16 cols = 8 KB/partition

CHUNKS = [1536, 1536, 2048, 3200]  # sum = 8320, runs 3-6.4 KB
OFFS = [sum(CHUNKS[:j]) for j in range(len(CHUNKS) + 1)]
N_JUNK = 30                   # N=128 warm-up matmuls bridging the DMA wait

# module-level knobs (test.py pokes these)
TRACE = os.environ.get("KERNEL_TRACE", "0") == "1"
USE_DT = os.environ.get("KERNEL_DT", "fp16")      # matmul operand dtype
OUT_DT = os.environ.get("KERNEL_OUT_DT", "fp16")  # device output dtype
LAST_RESULT = None

_CACHE = {}


def _build_m2(weight: np.ndarray) -> np.ndarray:
    """(EMB, 258) projection -> (W, EMB) causal-conv matrix, in float64."""
    k = np.arange(W_SIZE // 2 + 1, dtype=np.float64)   # 129
    w = np.arange(W_SIZE, dtype=np.float64)            # 256
    ang = 2.0 * np.pi * np.outer(k, w) / W_SIZE        # (129, 256)
    f = np.concatenate([np.cos(ang), -np.sin(ang)], axis=0)  # (258, 256)
    m2 = (weight.astype(np.float64) @ f).T             # (256, EMB)
    return np.ascontiguousarray(m2, dtype=np.float64)


def _round_fp22(a: np.ndarray) -> np.ndarray:
    """Round fp32 -> fp22 (e8m13, the TensorE f32r operand precision)."""
    u = np.ascontiguousarray(a, dtype=np.float32).view(np.uint32)
    u = (u + np.uint32(0x200)) & np.uint32(0xFFFFFC00)
    return u.view(np.float32)


def _build_program():
    from concourse import bacc, mybir, tile
    from concourse.ap import AP

    f32 = mybir.dt.float32
    fin = {
        "fp16": mybir.dt.float16,
        "bf16": mybir.dt.bfloat16,
        "f32r": mybir.dt.float32r,
        "f32": f32,
    }[USE_DT]
    fout = {"fp16": mybir.dt.float16, "bf16": mybir.dt.bfloat16, "f32": f32}[OUT_DT]
    add = mybir.AluOpType.add

    nc = bacc.Bacc(target_bir_lowering=False)
    xpad_h = nc.declare_dram_parameter("xpad", [B_PER, XP_LEN], fin, isOutput=False)
    # w2 pre-packed on host to the SBUF layout: w2[p, h*EMB+e] = M2[128h+p, e]
    w2_h = nc.declare_dram_parameter("w2", [128, 2 * EMB], fin, isOutput=False)
    # bias duplicated to cover a 2-bank (1024-wide) PSUM pair
    biasf_h = nc.declare_dram_parameter("biasf", [1, 2 * EMB], f32, isOutput=False)
    bias16_h = nc.declare_dram_parameter("bias16", [1, 2 * EMB], fout, isOutput=False)
    # partition-major output: out_dev[b, p, c*EMB+e] = out[b, 128c+p, e]
    out_h = nc.declare_dram_parameter(
        "out", [B_PER, 128, N_TILES * EMB], fout, isOutput=True
    )

    with tile.TileContext(nc) as tc:
        with (
            tc.tile_pool(name="hank", bufs=2) as hank_pool,
            tc.tile_pool(name="wpool", bufs=1) as w_pool,
            tc.tile_pool(name="cpool", bufs=1) as c_pool,
            tc.tile_pool(name="sup", bufs=4) as sup_pool,
            tc.tile_pool(name="psum", bufs=4, space="PSUM") as psum_pool,
        ):
            # PE pre-warm: short N=128 junk matmuls keep the PE busy from the
            # moment the engines are released, lifting the HAM 1.2 GHz cold
            # throttle while the input DMAs are still in flight.
            junk = c_pool.tile([128, 128], fin, tag="junk")
            nc.gpsimd.memset(junk[:, :], 0.0)
            ps_warm = psum_pool.tile([128, 2 * EMB], f32, tag="ps")
            for _ in range(N_JUNK):
                nc.tensor.matmul(
                    ps_warm[:, 0:128], junk[:, :], junk[:, :],
                    start=True, stop=True,
                )

            # sync ring: w01 then even chunks; scalar ring: odd chunks + bias
            w01 = w_pool.tile([128, 2 * EMB], fin, tag="w01")
            w0 = w01[:, 0:EMB]
            w1 = w01[:, EMB : 2 * EMB]
            nc.sync.dma_start(w01[:, :], w2_h[:, :])

            def alloc_hank(b):
                return [
                    hank_pool.tile([128, c], fin, tag=f"hk{j}", name=f"hk{j}_{b}")
                    for j, c in enumerate(CHUNKS)
                ]

            def load_chunk(b, tiles, j):
                eng = nc.sync if j % 2 == 0 else nc.scalar
                eng.dma_start(
                    tiles[j][:, :],
                    AP(xpad_h, b * XP_LEN + OFFS[j], [[1, 128], [1, CHUNKS[j]]]),
                )

            hanks = [alloc_hank(0), alloc_hank(1)]
            load_chunk(0, hanks[0], 0)   # sync: c0
            load_chunk(0, hanks[0], 1)   # scalar: c1

            bias_row = c_pool.tile([1, 2 * EMB], f32, tag="bias_row")
            nc.scalar.dma_start(bias_row[:, :], biasf_h[:, :])
            b16_row = c_pool.tile([1, 2 * EMB], fout, tag="b16_row")
            nc.scalar.dma_start(b16_row[:, :], bias16_h[:, :])
            bias_bc = c_pool.tile([128, 2 * EMB], f32, tag="bias_bc")
            nc.gpsimd.partition_broadcast(bias_bc[:, :], bias_row[:, :])
            bias_bc16 = c_pool.tile([128, 2 * EMB], fout, tag="bias_bc16")
            nc.gpsimd.partition_broadcast(bias_bc16[:, :], b16_row[:, :])

            load_chunk(0, hanks[0], 2)   # sync: c2
            load_chunk(0, hanks[0], 3)   # scalar: c3
            # batch-1 chunk loads are deferred into the batch-0 loop so they
            # don't steal ring bandwidth from the critical batch-0 head

            def hank_slice(b, c):
                """lhsT for column-block c (128 cols starting at 128*c)."""
                lo = 128 * c
                for j in range(len(CHUNKS)):
                    if lo + 128 <= OFFS[j + 1]:
                        off = lo - OFFS[j]
                        return hanks[b][j][:, off : off + 128]
                raise AssertionError(c)

            qglob = 0
            for b in range(B_PER):
                for g in range(N_SUP):
                    if b == 0 and 2 <= g <= 5:
                        load_chunk(1, hanks[1], g - 2)
                    sup = sup_pool.tile([128, SUP_W], fout)
                    for pq in range(PAIRS_PER_SUP):
                        ps = psum_pool.tile([128, 2 * EMB], f32)  # 2 banks
                        for h in range(2):
                            i = (g * PAIRS_PER_SUP + pq) * 2 + h
                            pslice = ps[:, h * EMB : (h + 1) * EMB]
                            nc.tensor.matmul(
                                pslice, hank_slice(b, i), w0,
                                start=True, stop=False,
                            )
                            nc.tensor.matmul(
                                pslice, hank_slice(b, i + 1), w1,
                                start=False, stop=True,
                            )
                        dst = sup[:, pq * 2 * EMB : (pq + 1) * 2 * EMB]
                        if qglob % 5 in (1, 2, 4) and qglob < 62:
                            # ACT evacuates; DVE applies bias in cheap 16-bit
                            # 2x mode. Offloads ~half of the evacuation work.
                            nc.scalar.copy(dst, ps[:, :])
                            nc.vector.tensor_tensor(dst, dst, bias_bc16[:, :], add)
                        else:
                            # single DVE op: evacuate + bias + cast
                            nc.vector.tensor_tensor(dst, ps[:, :], bias_bc[:, :], add)
                        qglob += 1
                    # store supertile: plain 2D slice in the partition-major
                    # layout -- one 8 KB contiguous run per partition
                    col0 = g * SUP_W
                    last = b == B_PER - 1 and g == N_SUP - 1
                    if last:
                        # split the final store across both rings to halve
                        # the end-of-kernel drain
                        half = SUP_W // 2
                        nc.sync.dma_start(
                            out_h[b, :, col0 : col0 + half], sup[:, 0:half]
                        )
                        nc.scalar.dma_start(
                            out_h[b, :, col0 + half : col0 + SUP_W],
                            sup[:, half:SUP_W],
                        )
                    else:
                        eng = nc.sync if g % 2 == 0 else nc.scalar
                        eng.dma_start(out_h[b, :, col0 : col0 + SUP_W], sup[:, :])

    nc.finalize()
    return nc


def _get_program():
    key = ("prog", USE_DT, OUT_DT)
    if key not in _CACHE:
        _CACHE[key] = _build_program()
    return _CACHE[key]


def kernel(x: np.ndarray, weight: np.ndarray, bias: np.ndarray) -> np.ndarray:
    global LAST_RESULT
    from concourse.bass_utils import run_bass_kernel_spmd

    x = np.asarray(x, dtype=np.float32)
    weight = np.asarray(weight, dtype=np.float32)
    bias = np.asarray(bias, dtype=np.float32)

    m2 = _build_m2(weight).astype(np.float32)
    xpad = np.zeros((B, XP_LEN), dtype=np.float32)
    xpad[:, PAD : PAD + T] = x
    # pack to the SBUF tile layout: w2[p, h*EMB+e] = M2[128h+p, e]
    w2_in = np.ascontiguousarray(
        m2.reshape(2, 128, EMB).transpose(1, 0, 2).reshape(128, 2 * EMB)
    )
    bias2 = np.ascontiguousarray(
        np.concatenate([bias, bias]).reshape(1, 2 * EMB).astype(np.float32)
    )

    import ml_dtypes

    np_in = {
        "fp16": np.float16,
        "bf16": ml_dtypes.bfloat16,
        "f32r": np.float32,
        "f32": np.float32,
    }[USE_DT]
    np_out = {"fp16": np.float16, "bf16": ml_dtypes.bfloat16, "f32": np.float32}[OUT_DT]
    if USE_DT == "f32r":
        w2_in = _round_fp22(w2_in)
        xpad = _round_fp22(xpad)
    else:
        w2_in = w2_in.astype(np_in)
        xpad = xpad.astype(np_in)

    nc = _get_program()
    in_maps = [
        {
            "xpad": np.ascontiguousarray(xpad[c * B_PER : (c + 1) * B_PER]),
            "w2": w2_in,
            "biasf": bias2,
            "bias16": np.ascontiguousarray(bias2.astype(np_out)),
        }
        for c in range(N_CORES)
    ]
    res = run_bass_kernel_spmd(nc, in_maps, list(range(N_CORES)), trace=TRACE)
    LAST_RESULT = res
    # un-permute the partition-major device layout:
    # out_dev[b, p, c*EMB+e] -> out[b, 128c+p, e]
    outs = []
    for c in range(N_CORES):
        od = np.asarray(res.results[c]["out"])  # [B_PER, 128, N_TILES*EMB]
        od = od.reshape(B_PER, 128, N_TILES, EMB).transpose(0, 2, 1, 3)
        outs.append(od.reshape(B_PER, T, EMB))
    out = np.concatenate(outs, axis=0)
    return np.ascontiguousarray(out.astype(np.float32))


# revision 8
# speedup vs baseline: 1.0600x; 1.0600x over previous
"""FFTEmbedding kernel for Trainium2 (8 NeuronCores, SPMD data-parallel over B).

Math: the reference computes, per (b, t):
    window = x_pad[b, t : t+W]                (causal window, W=256)
    spec   = rfft(window); feats = [spec.real, spec.imag]   (258)
    out    = feats @ weight.T + bias          (512)

The pipeline is linear in x, so it collapses to a causal 1-D convolution
with a precomputed (W=256, EMB=512) matrix:
    M2[w, e] = sum_k weight[e, k]*cos(2*pi*k*w/W) - weight[e, 129+k]*sin(2*pi*k*w/W)
    out[b, t, e] = sum_w x_pad[b, t+w] * M2[w, e] + bias[e]

Device mapping (per core: 2 batch rows, weights replicated). The kernel
is bound by the DMA fabric (16 SDMA engines shared by both HWDGE rings,
~25 GB/s each while streaming; every dma_start pays ~0.6-2 us of fixed
latency dominated by the completion receipt), so the design minimizes
DMA count, maximizes run length, and keeps the critical path alone on
one ring:
  * "mega-Hankel" SBUF image Hank[p, c] = x_pad[b, p+c] via overlapping
    DMA reads (partition p reads x_pad[b, p : p+...]); a 128-col slice
    IS the pre-transposed matmul lhsT. Batch 0 loads as 3 chunks on the
    sync ring ONLY (strict FIFO => the first chunk completes first);
    batch 1 loads as ONE 2.1 MB DMA on the scalar ring mid-stream.
  * per 128-t output tile i: psum[128, 512] = Hank[:, 128i:+128].T @ W0
    + Hank[:, 128(i+1):+128].T @ W1   (fp16 operands, fp32 PSUM accum).
    Two tiles share one 2-bank PSUM pair; evacuation fuses bias + fp16
    cast: 1/3 of pairs as a single DVE tensor_tensor, 2/3 as ACT copy +
    DVE 16-bit 2x add. The bias comes pre-broadcast from the HOST
    (gpsimd.partition_broadcast measured ~5 us AND holds the shared
    SBUF port that DVE 16-bit ops need -- never broadcast on device).
  * output uses a PARTITION-MAJOR DRAM layout out_dev[b, p, c*512+e] =
    out[b, 128c+p, e]: each 8-tile supertile store is a plain 2D slice
    with an 8 KB contiguous run per partition (measured 25.2 GB/s per
    packet vs 21.4 for the 1 KB runs of the natural layout). The host
    un-permutes with a cheap numpy reshape. Stores alternate rings; the
    final store is split across both rings to halve the drain.
  * PE warm-up: HAM clock gate needs ~3.4 us of sustained PE activity;
    N=128 junk matmuls bridge the input-DMA wait so the real stream
    starts at the warm clock rate.
  * output staged fp16 in SBUF, stored fp16 (halves the dominant HBM
    write stream); host upcasts to fp32. End-to-end rel err ~4e-4.
"""

import os
import sys

import numpy as np

_TRN_REPO = "/opt/trn_rl_repo"
if _TRN_REPO not in sys.path:
    sys.path.insert(0, _TRN_REPO)

B, T, W_SIZE, EMB = 16, 8192, 256, 512
N_CORES = 8
B_PER = B // N_CORES          # 2 batch rows per core
PAD = W_SIZE - 1              # 255 leading zeros
XP_LEN = T + PAD + 1          # 8448 (one trailing pad elem)
HANK_COLS = T + W_SIZE - 128  # 8320 mega-Hankel free dim
N_TILES = T // 128            # 64 output tiles of 128 t's per batch row
N_PAIRS = N_TILES // 2        # 32 PSUM pairs per batch row
PAIRS_PER_SUP = 4             # supertile = 4 pairs = 8 tiles = 1024 t rows
N_SUP = N_PAIRS // PAIRS_PER_SUP
SUP_W = 2 * PAIRS_PER_SUP * EMB  # 4096 fp16 cols = 8 KB/partition

CHUNKS = [1536, 2560, 4224]   # batch-0 ladder, sum = 8320, runs 3-8.3 KB
OFFS = [sum(CHUNKS[:j]) for j in range(len(CHUNKS) + 1)]
N_JUNK = 22                   # N=128 warm-up matmuls bridging the DMA wait

# module-level knobs (test.py pokes these)
TRACE = os.environ.get("KERNEL_TRACE", "0") == "1"
USE_DT = os.environ.get("KERNEL_DT", "fp16")      # matmul operand dtype
OUT_DT = os.environ.get("KERNEL_OUT_DT", "fp16")  # device output dtype
LAST_RESULT = None

_CACHE = {}


def _build_m2(weight: np.ndarray) -> np.ndarray:
    """(EMB, 258) projection -> (W, EMB) causal-conv matrix, in float64."""
    k = np.arange(W_SIZE // 2 + 1, dtype=np.float64)   # 129
    w = np.arange(W_SIZE, dtype=np.float64)            # 256
    ang = 2.0 * np.pi * np.outer(k, w) / W_SIZE        # (129, 256)
    f = np.concatenate([np.cos(ang), -np.sin(ang)], axis=0)  # (258, 256)
    m2 = (weight.astype(np.float64) @ f).T             # (256, EMB)
    return np.ascontiguousarray(m2, dtype=np.float64)


def _round_fp22(a: np.ndarray) -> np.ndarray:
    """Round fp32 -> fp22 (e8m13, the TensorE f32r operand precision)."""
    u = np.ascontiguousarray(a, dtype=np.float32).view(np.uint32)
    u = (u + np.uint32(0x200)) & np.uint32(0xFFFFFC00)
    return u.view(np.float32)


def _build_program():
    from concourse import bacc, mybir, tile
    from concourse.ap import AP

    f32 = mybir.dt.float32
    fin = {
        "fp16": mybir.dt.float16,
        "bf16": mybir.dt.bfloat16,
        "f32r": mybir.dt.float32r,
        "f32": f32,
    }[USE_DT]
    fout = {"fp16": mybir.dt.float16, "bf16": mybir.dt.bfloat16, "f32": f32}[OUT_DT]
    add = mybir.AluOpType.add

    nc = bacc.Bacc(target_bir_lowering=False)
    xpad_h = nc.declare_dram_parameter("xpad", [B_PER, XP_LEN], fin, isOutput=False)
    # w2 pre-packed on host to the SBUF layout: w2[p, h*EMB+e] = M2[128h+p, e]
    w2_h = nc.declare_dram_parameter("w2", [128, 2 * EMB], fin, isOutput=False)
    # bias pre-broadcast on host to all 128 partitions, 2x along e to
    # cover a 2-bank (1024-wide) PSUM pair; fout so the DVE 16-bit 2x
    # add can consume it directly
    biasbc_h = nc.declare_dram_parameter(
        "biasbc", [128, 2 * EMB], fout, isOutput=False
    )
    # partition-major output: out_dev[b, p, c*EMB+e] = out[b, 128c+p, e]
    out_h = nc.declare_dram_parameter(
        "out", [B_PER, 128, N_TILES * EMB], fout, isOutput=True
    )

    with tile.TileContext(nc) as tc:
        with (
            tc.tile_pool(name="hank", bufs=1) as hank_pool,
            tc.tile_pool(name="wpool", bufs=1) as w_pool,
            tc.tile_pool(name="cpool", bufs=1) as c_pool,
            tc.tile_pool(name="sup", bufs=4) as sup_pool,
            tc.tile_pool(name="psum", bufs=4, space="PSUM") as psum_pool,
        ):
            # PE pre-warm: N=128 junk matmuls keep the PE busy from engine
            # release, lifting the HAM 1.2 GHz cold throttle while the
            # input DMAs are in flight.
            junk = c_pool.tile([128, 128], fin, tag="junk")
            nc.gpsimd.memset(junk[:, :], 0.0)
            ps_warm = psum_pool.tile([128, 2 * EMB], f32, tag="ps")
            for _ in range(N_JUNK):
                nc.tensor.matmul(
                    ps_warm[:, 0:128], junk[:, :], junk[:, :],
                    start=True, stop=True,
                )

            # critical path rides the sync ring ALONE, strict FIFO:
            # w2 -> hank c0 -> bias -> hank c1 -> hank c2
            w01 = w_pool.tile([128, 2 * EMB], fin, tag="w01")
            w0 = w01[:, 0:EMB]
            w1 = w01[:, EMB : 2 * EMB]
            nc.sync.dma_start(w01[:, :], w2_h[:, :])

            hk0 = [
                hank_pool.tile([128, c], fin, tag=f"hk{j}", name=f"hk{j}")
                for j, c in enumerate(CHUNKS)
            ]
            hk1 = hank_pool.tile([128, HANK_COLS], fin, tag="hkb1")

            def load_chunk0(j):
                nc.sync.dma_start(
                    hk0[j][:, :],
                    AP(xpad_h, OFFS[j], [[1, 128], [1, CHUNKS[j]]]),
                )

            load_chunk0(0)
            biasbc = c_pool.tile([128, 2 * EMB], fout, tag="biasbc")
            nc.sync.dma_start(biasbc[:, :], biasbc_h[:, :])
            load_chunk0(1)
            load_chunk0(2)

            def hank_slice(b, c):
                """lhsT for column-block c (128 cols starting at 128*c)."""
                lo = 128 * c
                if b == 1:
                    return hk1[:, lo : lo + 128]
                for j in range(len(CHUNKS)):
                    if lo + 128 <= OFFS[j + 1]:
                        off = lo - OFFS[j]
                        return hk0[j][:, off : off + 128]
                raise AssertionError(c)

            qglob = 0
            for b in range(B_PER):
                for g in range(N_SUP):
                    if b == 0 and g == 2:
                        # batch-1 Hankel: ONE big DMA on the scalar ring,
                        # interleaving with stores long before it's needed
                        nc.scalar.dma_start(
                            hk1[:, :],
                            AP(xpad_h, XP_LEN, [[1, 128], [1, HANK_COLS]]),
                        )
                    sup = sup_pool.tile([128, SUP_W], fout)
                    for pq in range(PAIRS_PER_SUP):
                        ps = psum_pool.tile([128, 2 * EMB], f32)  # 2 banks
                        for h in range(2):
                            i = (g * PAIRS_PER_SUP + pq) * 2 + h
                            pslice = ps[:, h * EMB : (h + 1) * EMB]
                            nc.tensor.matmul(
                                pslice, hank_slice(b, i), w0,
                                start=True, stop=False,
                            )
                            nc.tensor.matmul(
                                pslice, hank_slice(b, i + 1), w1,
                                start=False, stop=True,
                            )
                        dst = sup[:, pq * 2 * EMB : (pq + 1) * 2 * EMB]
                        if qglob % 3 == 0:
                            # single DVE op: evacuate + bias + cast
                            nc.vector.tensor_tensor(dst, ps[:, :], biasbc[:, :], add)
                        else:
                            # ACT evacuates; DVE applies bias in 16-bit 2x
                            nc.scalar.copy(dst, ps[:, :])
                            nc.vector.tensor_tensor(dst, dst, biasbc[:, :], add)
                        qglob += 1
                    # store supertile: plain 2D slice in the partition-major
                    # layout -- one 8 KB contiguous run per partition
                    col0 = g * SUP_W
                    last = b == B_PER - 1 and g == N_SUP - 1
                    if last:
                        half = SUP_W // 2
                        nc.sync.dma_start(
                            out_h[b, :, col0 : col0 + half], sup[:, 0:half]
                        )
                        nc.scalar.dma_start(
                            out_h[b, :, col0 + half : col0 + SUP_W],
                            sup[:, half:SUP_W],
                        )
                    else:
                        eng = nc.sync if g % 2 == 0 else nc.scalar
                        eng.dma_start(out_h[b, :, col0 : col0 + SUP_W], sup[:, :])

    nc.finalize()
    return nc


def _get_program():
    key = ("prog", USE_DT, OUT_DT)
    if key not in _CACHE:
        _CACHE[key] = _build_program()
    return _CACHE[key]


def kernel(x: np.ndarray, weight: np.ndarray, bias: np.ndarray) -> np.ndarray:
    global LAST_RESULT
    from concourse.bass_utils import run_bass_kernel_spmd

    x = np.asarray(x, dtype=np.float32)
    weight = np.asarray(weight, dtype=np.float32)
    bias = np.asarray(bias, dtype=np.float32)

    m2 = _build_m2(weight).astype(np.float32)
    xpad = np.zeros((B, XP_LEN), dtype=np.float32)
    xpad[:, PAD : PAD + T] = x
    # pack to the SBUF tile layout: w2[p, h*EMB+e] = M2[128h+p, e]
    w2_in = np.ascontiguousarray(
        m2.reshape(2, 128, EMB).transpose(1, 0, 2).reshape(128, 2 * EMB)
    )
    bias2 = np.concatenate([bias, bias]).reshape(1, 2 * EMB).astype(np.float32)

    import ml_dtypes

    np_in = {
        "fp16": np.float16,
        "bf16": ml_dtypes.bfloat16,
        "f32r": np.float32,
        "f32": np.float32,
    }[USE_DT]
    np_out = {"fp16": np.float16, "bf16": ml_dtypes.bfloat16, "f32": np.float32}[OUT_DT]
    if USE_DT == "f32r":
        w2_in = _round_fp22(w2_in)
        xpad = _round_fp22(xpad)
    else:
        w2_in = w2_in.astype(np_in)
        xpad = xpad.astype(np_in)
    # bias pre-broadcast to all partitions on the host
    biasbc_in = np.ascontiguousarray(np.tile(bias2.astype(np_out), (128, 1)))

    nc = _get_program()
    in_maps = [
        {
            "xpad": np.ascontiguousarray(xpad[c * B_PER : (c + 1) * B_PER]),
            "w2": w2_in,
            "biasbc": biasbc_in,
        }
        for c in range(N_CORES)
    ]
    res = run_bass_kernel_spmd(nc, in_maps, list(range(N_CORES)), trace=TRACE)
    LAST_RESULT = res
    # un-permute the partition-major device layout:
    # out_dev[b, p, c*EMB+e] -> out[b, 128c+p, e]
    outs = []
    for c in range(N_CORES):
        od = np.asarray(res.results[c]["out"])  # [B_PER, 128, N_TILES*EMB]
        od = od.reshape(B_PER, 128, N_TILES, EMB).transpose(0, 2, 1, 3)
        outs.append(od.reshape(B_PER, T, EMB))
    out = np.concatenate(outs, axis=0)
    return np.ascontiguousarray(out.astype(np.float32))
